# revision 21
# baseline (speedup 1.0000x reference)
"""Pairwise cosine-similarity adjacency (exp(-0.5 * cos_sim)) on 8 trn2 cores.

Input : x [4, 4096, 512] fp32
Output: exp(-0.5 * (xn @ xn.T)) per batch -> [4, 4096, 4096] fp32,
        xn = x / max(||x||_row, 1e-8)

Sharding (symmetry-aware): batch b = core // 2; even core owns rows
0..2047, odd core rows 2048..4095 (cross = the other half, odd-core cross
rotated by 1024 so the quarter-block cover is a triangle cover).  The
device computes, per core (local coords):
  - dtop rows 0..1023  x cols m*128..2047 (Q00 upper-triangle 128-blocks
    + all of Q01); garbage left in the skipped lower-left of Q00
  - dbot rows 0..1023  x cols mm*128..1023 of Q11 (upper triangle)
  - outc 2048 rows x 1024 cols (Q02 / Q13 via the cross side)
Host mirrors every skipped block from its transpose.

v4: the host pre-normalizes rows (O(N*D)), casts to fp8e4, and packs the
operands TRANSPOSED in the exact fp8 DoubleRow SBUF layout
[128(p), 2(P), 2(i), 2048(col)] with contraction index kd = 256P+128i+p.
The device is pure GEMM: 2 x 1MB loads, 144 fp8 DoubleRow matmuls
(K=256/instr, one accumulation group per PSUM bank, chunks <=512
bank-aligned), Exp(scale=-0.5) -> bf16, per-segment DMAs out.  All the
O(N^2 D) matmul work, the O(N^2) exp, and the full output remain on
device; host only preps inputs and mirrors the symmetric half.
"""
import sys

sys.path.insert(0, '/opt/trn_rl_repo')

import numpy as np
import ml_dtypes

B, N, D = 4, 4096, 512
N_CORES = 8
R = N // 2      # 2048 own rows per core
Q = N // 4      # 1024 quarter-block size
EPS = 1e-8

_compiled = {}


def _build():
    import concourse.mybir as mybir
    import concourse.tile as tile
    from concourse import bacc

    fp32 = mybir.dt.float32
    bf16 = mybir.dt.bfloat16
    fp8 = mybir.dt.float8e4
    DR = mybir.MatmulPerfMode.DoubleRow

    nc = bacc.Bacc(trn_type="TRN2", target_bir_lowering=False, debug=False,
                   num_devices=N_CORES)
    # pre-transposed fp8 DoubleRow operands: [p, P, i, col],
    # kd = 256*P + 128*i + p   (side 0 = own rows, 1 = cross rows)
    xnt = [nc.dram_tensor(f"xnt{s}", [128, 2, 2, R], fp8,
                          kind="ExternalInput") for s in range(2)]
    dtop = nc.dram_tensor("dtop", [Q, 2 * Q], bf16, kind="ExternalOutput")
    dbot = nc.dram_tensor("dbot", [Q, Q], bf16, kind="ExternalOutput")
    # outc stored [p, m, col]: logical row m*128+p  (host untransposes)
    outc = nc.dram_tensor("outc", [128, 16, Q], bf16, kind="ExternalOutput")

    with tile.TileContext(nc) as tc:
        with tc.tile_pool(name="store", bufs=1) as store, \
             tc.tile_pool(name="psum", bufs=2, space="PSUM") as psum_pool, \
             tc.tile_pool(name="p2out", bufs=4) as p2out:

            xnT4 = [store.tile([128, 2, 2, R], fp8, name=f"xnT4_{s}")
                    for s in range(2)]
            # own side first, in 512-col slices, so the first fills'
            # chunk-level deps are satisfied as early as possible
            eng_in = [nc.sync, nc.gpsimd, nc.sync, nc.gpsimd]
            for k, c0 in enumerate(range(0, R, 512)):
                eng_in[k].dma_start(xnT4[0][:, :, :, c0:c0 + 512],
                                    xnt[0].ap()[:, :, :, c0:c0 + 512])
            for k, c0 in enumerate(range(0, R, Q)):
                eng_in[k].dma_start(xnT4[1][:, :, :, c0:c0 + Q],
                                    xnt[1].ap()[:, :, :, c0:c0 + Q])

            # ---- fill plan (exact triangle cover) ----
            # segment: (m, s, c0, w, dst, drow, dcol); PSUM footprint of a
            # segment is ceil(w/512)*512 (one matmul accum group per bank).
            def seg_top(m):     # Q00 triangle row + Q01, merged
                w = 2048 - m * 128
                return (m, 0, m * 128, w, dtop, m * 128, m * 128)

            def seg_bot(m):     # Q11 triangle row
                mm = m - 8
                w = 1024 - mm * 128
                return (m, 0, 1024 + mm * 128, w, dbot, mm * 128, mm * 128)

            def seg_outc(m):
                return (m, 1, (m // 8) * 1024, 1024, outc, m, 0)

            # pack 512-multiple segs first so most fills are one gap-free
            # Exp run; interleave the 2-run fills with cheap outc fills
            oc = [[seg_outc(m), seg_outc(m + 1)] for m in range(0, 16, 2)]
            fills = [[seg_top(0)], [seg_top(1)], [seg_top(2)], [seg_top(3)],
                     [seg_top(4), seg_bot(13)], [seg_bot(12), seg_top(5)],
                     [seg_bot(8), seg_bot(9)], [seg_top(6), seg_bot(14)],
                     [seg_top(7), seg_bot(15)],
                     [seg_bot(10), seg_bot(11)]] + oc

            for f, segs in enumerate(fills):
                acc = psum_pool.tile([128, 2048], fp32, tag="ps",
                                     name=f"acc_{f}")
                offs, off = [], 0
                for (m, s, c0, w, _, _, _) in segs:
                    offs.append(off)
                    off += -(-w // 512) * 512
                for P in range(2):
                    for (m, s, c0, w, _, _, _), so in zip(segs, offs):
                        for o in range(0, w, 512):
                            wc = min(512, w - o)
                            nc.tensor.matmul(
                                acc[:, so + o:so + o + wc],
                                xnT4[0][:, P, :, m * 128:(m + 1) * 128],
                                xnT4[s][:, P, :, c0 + o:c0 + o + wc],
                                start=(P == 0), stop=(P == 1),
                                perf_mode=DR)
                ot = p2out.tile([128, 2048], bf16, tag="ot", name=f"ot_{f}")
                # Exp over gap-free runs of segments
                run_s, run_e = None, None
                for (m, s, c0, w, _, _, _), so in zip(segs, offs):
                    if run_e == so:
                        run_e = so + w
                    else:
                        if run_s is not None:
                            nc.scalar.activation(
                                ot[:, run_s:run_e], acc[:, run_s:run_e],
                                mybir.ActivationFunctionType.Exp, scale=-0.5)
                        run_s, run_e = so, so + w
                nc.scalar.activation(ot[:, run_s:run_e], acc[:, run_s:run_e],
                                     mybir.ActivationFunctionType.Exp,
                                     scale=-0.5)
                eng = [nc.sync, nc.gpsimd][f % 2]
                if segs[0][4] is outc:
                    # one DMA for the whole fill: [p, 2(m), 1024] layout
                    m0 = segs[0][0]
                    eng.dma_start(outc.ap()[:, m0:m0 + 2, :],
                                  ot[:, 0:2048])
                else:
                    for (m, s, c0, w, dst, drow, dcol), so in zip(segs, offs):
                        eng.dma_start(
                            dst.ap()[drow:drow + 128, dcol:dcol + w],
                            ot[:, so:so + w])

    nc.compile()
    return nc


def _pack(xn_rows):
    """[2048, 512] fp8 -> [128, 2, 2, 2048] DoubleRow layout, contiguous."""
    # xnT[d, col]; d = 256*P + 128*i + p  ->  [p, P, i, col]
    t = np.ascontiguousarray(
        xn_rows.T.reshape(2, 2, 128, R).transpose(2, 0, 1, 3))
    return t


def _in_maps(x):
    norm = np.sqrt(np.sum(x.astype(np.float64) ** 2, axis=-1, keepdims=True))
    xn = (x / np.maximum(norm, EPS)).astype(np.float32)
    xn8 = xn.astype(ml_dtypes.float8_e4m3fn)
    maps = []
    for c in range(N_CORES):
        b = c // 2
        xb = xn8[b]
        if c % 2 == 0:
            own, cross = xb[0:R], xb[R:N]
        else:
            own = xb[R:N]
            cross = np.concatenate([xb[Q:2 * Q], xb[0:Q]])
        maps.append({"xnt0": _pack(own), "xnt1": _pack(cross)})
    return maps


def _assemble(results, out):
    for c in range(N_CORES):
        b, odd = c // 2, c % 2
        o = out[b]
        r0 = odd * 2 * Q                  # own-row offset: 0 or 2048
        dtop = np.asarray(results[c]["dtop"]).astype(np.float32)
        dbot = np.asarray(results[c]["dbot"]).astype(np.float32)
        outc = np.asarray(results[c]["outc"]).astype(np.float32) \
            .transpose(1, 0, 2).reshape(2 * Q, Q)
        o[r0:r0 + Q, r0:r0 + 2 * Q] = dtop
        o[r0 + Q:r0 + 2 * Q, r0 + Q:r0 + 2 * Q] = dbot
        # mirror skipped lower-left 128-blocks inside the diagonal blocks
        for base in (r0, r0 + Q):
            for mm in range(1, 8):
                rr = base + mm * 128
                for cc in range(mm):
                    cb = base + cc * 128
                    o[rr:rr + 128, cb:cb + 128] = \
                        o[cb:cb + 128, rr:rr + 128].T
        o[r0 + Q:r0 + 2 * Q, r0:r0 + Q] = dtop[:, Q:2 * Q].T
        # cross cols: even core -> [2048.., 3072..]; odd -> [1024.., 0..]
        ccol = [2 * Q, 3 * Q] if not odd else [Q, 0]
        for half in range(2):
            blk = outc[half * Q:(half + 1) * Q]
            rr = r0 + half * Q
            cc = ccol[half]
            o[rr:rr + Q, cc:cc + Q] = blk
            o[cc:cc + Q, rr:rr + Q] = blk.T
    return out


def kernel(x: np.ndarray) -> np.ndarray:
    from concourse.bass_utils import run_bass_kernel_spmd

    x = np.asarray(x, dtype=np.float32)
    assert x.shape == (B, N, D)

    if "nc" not in _compiled:
        _compiled["nc"] = _build()
    nc = _compiled["nc"]

    res = run_bass_kernel_spmd(nc, _in_maps(x), list(range(N_CORES)))
    out = np.empty((B, N, N), dtype=np.float32)
    return _assemble([res.results[c] for c in range(N_CORES)], out)


# revision 23
# speedup vs baseline: 1.0521x; 1.0521x over previous
"""Pairwise cosine-similarity adjacency (exp(-0.5 * cos_sim)) on 8 trn2 cores.

Input : x [4, 4096, 512] fp32
Output: exp(-0.5 * (xn @ xn.T)) per batch -> [4, 4096, 4096] fp32,
        xn = x / max(||x||_row, 1e-8)

Sharding (symmetry-aware): batch b = core // 2; even core owns rows
0..2047, odd core rows 2048..4095 (cross = the other half, odd-core cross
rotated by 1024 so the quarter-block cover is a triangle cover).  The
device computes, per core (local coords):
  - dtop rows 0..1023  x cols m*128..2047 (Q00 upper-triangle 128-blocks
    + all of Q01); garbage left in the skipped lower-left of Q00
  - dbot rows 0..1023  x cols mm*128..1023 of Q11 (upper triangle)
  - outc 2048 rows x 1024 cols (Q02 / Q13 via the cross side)
Host mirrors every skipped block from its transpose.

v4: the host pre-normalizes rows (O(N*D)), casts to fp8e4, and packs the
operands TRANSPOSED in the exact fp8 DoubleRow SBUF layout
[128(p), 2(P), 2(i), 2048(col)] with contraction index kd = 256P+128i+p.
The device is pure GEMM: 2 x 1MB loads, 144 fp8 DoubleRow matmuls
(K=256/instr, one accumulation group per PSUM bank, chunks <=512
bank-aligned), Exp(scale=-0.5) -> bf16, per-segment DMAs out.  All the
O(N^2 D) matmul work, the O(N^2) exp, and the full output remain on
device; host only preps inputs and mirrors the symmetric half.
"""
import sys

sys.path.insert(0, '/opt/trn_rl_repo')

import numpy as np
import ml_dtypes

B, N, D = 4, 4096, 512
N_CORES = 8
R = N // 2      # 2048 own rows per core
Q = N // 4      # 1024 quarter-block size
EPS = 1e-8

_compiled = {}


def _build():
    import concourse.mybir as mybir
    import concourse.tile as tile
    from concourse import bacc

    fp32 = mybir.dt.float32
    bf16 = mybir.dt.bfloat16
    fp8 = mybir.dt.float8e4
    DR = mybir.MatmulPerfMode.DoubleRow

    nc = bacc.Bacc(trn_type="TRN2", target_bir_lowering=False, debug=False,
                   num_devices=N_CORES)
    # pre-transposed fp8 DoubleRow operands: [p, P, i, col],
    # kd = 256*P + 128*i + p   (side 0 = own rows, 1 = cross rows)
    xnt = [nc.dram_tensor(f"xnt{s}", [128, 2, 2, R], fp8,
                          kind="ExternalInput") for s in range(2)]
    dtop = nc.dram_tensor("dtop", [Q, 2 * Q], bf16, kind="ExternalOutput")
    dbot = nc.dram_tensor("dbot", [Q, Q], bf16, kind="ExternalOutput")
    # outc stored [p, m, col]: logical row m*128+p  (host untransposes)
    outc = nc.dram_tensor("outc", [128, 16, Q], bf16, kind="ExternalOutput")

    with tile.TileContext(nc) as tc:
        with tc.tile_pool(name="store", bufs=1) as store, \
             tc.tile_pool(name="psum", bufs=2, space="PSUM") as psum_pool, \
             tc.tile_pool(name="p2out", bufs=4) as p2out:

            xnT4 = [store.tile([128, 2, 2, R], fp8, name=f"xnT4_{s}")
                    for s in range(2)]
            # own side first, in 512-col slices, so the first fills'
            # chunk-level deps are satisfied as early as possible
            for c0 in range(0, R, 512):
                nc.sync.dma_start(xnT4[0][:, :, :, c0:c0 + 512],
                                  xnt[0].ap()[:, :, :, c0:c0 + 512])
            for c0 in range(0, R, Q):
                nc.sync.dma_start(xnT4[1][:, :, :, c0:c0 + Q],
                                  xnt[1].ap()[:, :, :, c0:c0 + Q])

            # ---- fill plan (exact triangle cover) ----
            # segment: (m, s, c0, w, dst, drow, dcol); PSUM footprint of a
            # segment is ceil(w/512)*512 (one matmul accum group per bank).
            def seg_top(m):     # Q00 triangle row + Q01, merged
                w = 2048 - m * 128
                return (m, 0, m * 128, w, dtop, m * 128, m * 128)

            def seg_bot(m):     # Q11 triangle row
                mm = m - 8
                w = 1024 - mm * 128
                return (m, 0, 1024 + mm * 128, w, dbot, mm * 128, mm * 128)

            def seg_outc(m):
                return (m, 1, (m // 8) * 1024, 1024, outc, m, 0)

            # pack 512-multiple segs first so most fills are one gap-free
            # Exp run; interleave the 2-run fills with cheap outc fills
            oc = [[seg_outc(m), seg_outc(m + 1)] for m in range(0, 16, 2)]
            fills = [[seg_top(0)], [seg_top(1)], [seg_top(2)], [seg_top(3)],
                     [seg_top(4), seg_bot(13)], [seg_bot(12), seg_top(5)],
                     [seg_bot(8), seg_bot(9)], [seg_top(6), seg_bot(14)],
                     [seg_top(7), seg_bot(15)],
                     [seg_bot(10), seg_bot(11)]] + oc

            for f, segs in enumerate(fills):
                acc = psum_pool.tile([128, 2048], fp32, tag="ps",
                                     name=f"acc_{f}")
                offs, off = [], 0
                for (m, s, c0, w, _, _, _) in segs:
                    offs.append(off)
                    off += -(-w // 512) * 512
                for P in range(2):
                    for (m, s, c0, w, _, _, _), so in zip(segs, offs):
                        for o in range(0, w, 512):
                            wc = min(512, w - o)
                            nc.tensor.matmul(
                                acc[:, so + o:so + o + wc],
                                xnT4[0][:, P, :, m * 128:(m + 1) * 128],
                                xnT4[s][:, P, :, c0 + o:c0 + o + wc],
                                start=(P == 0), stop=(P == 1),
                                perf_mode=DR)
                ot = p2out.tile([128, 2048], bf16, tag="ot", name=f"ot_{f}")
                # Exp over gap-free runs of segments
                run_s, run_e = None, None
                for (m, s, c0, w, _, _, _), so in zip(segs, offs):
                    if run_e == so:
                        run_e = so + w
                    else:
                        if run_s is not None:
                            nc.scalar.activation(
                                ot[:, run_s:run_e], acc[:, run_s:run_e],
                                mybir.ActivationFunctionType.Exp, scale=-0.5)
                        run_s, run_e = so, so + w
                nc.scalar.activation(ot[:, run_s:run_e], acc[:, run_s:run_e],
                                     mybir.ActivationFunctionType.Exp,
                                     scale=-0.5)
                if segs[0][4] is outc:
                    # one DMA for the whole fill: [p, 2(m), 1024] layout
                    m0 = segs[0][0]
                    nc.sync.dma_start(outc.ap()[:, m0:m0 + 2, :],
                                      ot[:, 0:2048])
                else:
                    for (m, s, c0, w, dst, drow, dcol), so in zip(segs, offs):
                        nc.sync.dma_start(
                            dst.ap()[drow:drow + 128, dcol:dcol + w],
                            ot[:, so:so + w])

    nc.compile()
    return nc


def _pack(xn_rows):
    """[2048, 512] fp8 -> [128, 2, 2, 2048] DoubleRow layout, contiguous."""
    # xnT[d, col]; d = 256*P + 128*i + p  ->  [p, P, i, col]
    t = np.ascontiguousarray(
        xn_rows.T.reshape(2, 2, 128, R).transpose(2, 0, 1, 3))
    return t


def _in_maps(x):
    norm = np.sqrt(np.sum(x.astype(np.float64) ** 2, axis=-1, keepdims=True))
    xn = (x / np.maximum(norm, EPS)).astype(np.float32)
    xn8 = xn.astype(ml_dtypes.float8_e4m3fn)
    maps = []
    for c in range(N_CORES):
        b = c // 2
        xb = xn8[b]
        if c % 2 == 0:
            own, cross = xb[0:R], xb[R:N]
        else:
            own = xb[R:N]
            cross = np.concatenate([xb[Q:2 * Q], xb[0:Q]])
        maps.append({"xnt0": _pack(own), "xnt1": _pack(cross)})
    return maps


def _assemble(results, out):
    for c in range(N_CORES):
        b, odd = c // 2, c % 2
        o = out[b]
        r0 = odd * 2 * Q                  # own-row offset: 0 or 2048
        dtop = np.asarray(results[c]["dtop"]).astype(np.float32)
        dbot = np.asarray(results[c]["dbot"]).astype(np.float32)
        outc = np.asarray(results[c]["outc"]).astype(np.float32) \
            .transpose(1, 0, 2).reshape(2 * Q, Q)
        o[r0:r0 + Q, r0:r0 + 2 * Q] = dtop
        o[r0 + Q:r0 + 2 * Q, r0 + Q:r0 + 2 * Q] = dbot
        # mirror skipped lower-left 128-blocks inside the diagonal blocks
        for base in (r0, r0 + Q):
            for mm in range(1, 8):
                rr = base + mm * 128
                for cc in range(mm):
                    cb = base + cc * 128
                    o[rr:rr + 128, cb:cb + 128] = \
                        o[cb:cb + 128, rr:rr + 128].T
        o[r0 + Q:r0 + 2 * Q, r0:r0 + Q] = dtop[:, Q:2 * Q].T
        # cross cols: even core -> [2048.., 3072..]; odd -> [1024.., 0..]
        ccol = [2 * Q, 3 * Q] if not odd else [Q, 0]
        for half in range(2):
            blk = outc[half * Q:(half + 1) * Q]
            rr = r0 + half * Q
            cc = ccol[half]
            o[rr:rr + Q, cc:cc + Q] = blk
            o[cc:cc + Q, rr:rr + Q] = blk.T
    return out


def kernel(x: np.ndarray) -> np.ndarray:
    from concourse.bass_utils import run_bass_kernel_spmd

    x = np.asarray(x, dtype=np.float32)
    assert x.shape == (B, N, D)

    if "nc" not in _compiled:
        _compiled["nc"] = _build()
    nc = _compiled["nc"]

    res = run_bass_kernel_spmd(nc, _in_maps(x), list(range(N_CORES)))
    out = np.empty((B, N, N), dtype=np.float32)
    return _assemble([res.results[c] for c in range(N_CORES)], out)


# revision 27
# speedup vs baseline: 1.1295x; 1.0735x over previous
"""Pairwise cosine-similarity adjacency (exp(-0.5 * cos_sim)) on 8 trn2 cores.

Input : x [4, 4096, 512] fp32
Output: exp(-0.5 * (xn @ xn.T)) per batch -> [4, 4096, 4096] fp32,
        xn = x / max(||x||_row, 1e-8)

Sharding (symmetry-aware): batch b = core // 2; even core owns rows
0..2047, odd core rows 2048..4095 (cross = the other half, odd-core cross
rotated by 1024 so the quarter-block cover is a triangle cover).  The
device computes, per core (local coords):
  - dtop rows 0..1023  x cols m*128..2047 (Q00 upper-triangle 128-blocks
    + all of Q01); garbage left in the skipped lower-left of Q00
  - dbot rows 0..1023  x cols mm*128..1023 of Q11 (upper triangle)
  - outc 2048 rows x 1024 cols (Q02 / Q13 via the cross side)
Host mirrors every skipped block from its transpose.

v4: the host pre-normalizes rows (O(N*D)), casts to fp8e4, and packs the
operands TRANSPOSED in the exact fp8 DoubleRow SBUF layout
[128(p), 2(P), 2(i), 2048(col)] with contraction index kd = 256P+128i+p.
The device is pure GEMM: 2 x 1MB loads, 144 fp8 DoubleRow matmuls
(K=256/instr, one accumulation group per PSUM bank, chunks <=512
bank-aligned), Exp(scale=-0.5) -> bf16, per-segment DMAs out.  All the
O(N^2 D) matmul work, the O(N^2) exp, and the full output remain on
device; host only preps inputs and mirrors the symmetric half.
"""
import sys

sys.path.insert(0, '/opt/trn_rl_repo')

import numpy as np
import ml_dtypes

B, N, D = 4, 4096, 512
N_CORES = 8
R = N // 2      # 2048 own rows per core
Q = N // 4      # 1024 quarter-block size
EPS = 1e-8

_compiled = {}


def _build():
    import concourse.mybir as mybir
    import concourse.tile as tile
    from concourse import bacc

    fp32 = mybir.dt.float32
    bf16 = mybir.dt.bfloat16
    fp8 = mybir.dt.float8e4
    DR = mybir.MatmulPerfMode.DoubleRow

    nc = bacc.Bacc(trn_type="TRN2", target_bir_lowering=False, debug=False,
                   num_devices=N_CORES)
    # pre-transposed fp8 DoubleRow operands: [p, P, i, col],
    # kd = 256*P + 128*i + p   (side 0 = own rows, 1 = cross rows)
    xnt = [nc.dram_tensor(f"xnt{s}", [128, 2, 2, R], fp8,
                          kind="ExternalInput") for s in range(2)]
    dtop = nc.dram_tensor("dtop", [Q, 2 * Q], bf16, kind="ExternalOutput")
    dbot = nc.dram_tensor("dbot", [Q, Q], bf16, kind="ExternalOutput")
    # outc stored [p, m, col]: logical row m*128+p  (host untransposes)
    outc = nc.dram_tensor("outc", [128, 16, Q], bf16, kind="ExternalOutput")

    with tile.TileContext(nc) as tc:
        with tc.tile_pool(name="store", bufs=1) as store, \
             tc.tile_pool(name="psum", bufs=2, space="PSUM") as psum_pool, \
             tc.tile_pool(name="p2out", bufs=4) as p2out:

            xnT4 = [store.tile([128, 2, 2, R], fp8, name=f"xnT4_{s}")
                    for s in range(2)]
            # own side first, in 512-col slices, so the first fills'
            # chunk-level deps are satisfied as early as possible
            for c0 in range(0, R, 512):
                nc.sync.dma_start(xnT4[0][:, :, :, c0:c0 + 512],
                                  xnt[0].ap()[:, :, :, c0:c0 + 512])
            for c0 in range(0, R, Q):
                nc.sync.dma_start(xnT4[1][:, :, :, c0:c0 + Q],
                                  xnt[1].ap()[:, :, :, c0:c0 + Q])

            # ---- fill plan (exact triangle cover) ----
            # segment: (m, s, c0, w, dst, drow, dcol); PSUM footprint of a
            # segment is ceil(w/512)*512 (one matmul accum group per bank).
            def seg_top(m, ext=0):   # Q00 triangle row + Q01, merged
                # ext: extend left into the host-mirrored garbage region
                # to make w a 512-multiple (merges the fill's Exp runs)
                c0 = m * 128 - ext
                return (m, 0, c0, 2048 - c0, dtop, m * 128, c0)

            def seg_bot(m, ext=0):   # Q11 triangle row
                mm = m - 8
                c0 = mm * 128 - ext
                return (m, 0, 1024 + c0, 1024 - c0, dbot, mm * 128, c0)

            def seg_outc(m):
                return (m, 1, (m // 8) * 1024, 1024, outc, m, 0)

            # pack 512-multiple segs first so most fills are one gap-free
            # Exp run; interleave the 2-run fills with cheap outc fills
            oc = [[seg_outc(m), seg_outc(m + 1)] for m in range(0, 16, 2)]
            fills = [[seg_top(0)], [seg_top(1)], [seg_top(2)], [seg_top(3)],
                     [seg_top(4), seg_bot(13)], [seg_bot(12), seg_top(5)],
                     [seg_bot(8), seg_bot(9)],
                     [seg_top(6, ext=256), seg_bot(14)],
                     [seg_top(7, ext=384), seg_bot(15)],
                     [seg_bot(10, ext=256), seg_bot(11, ext=384)]] + oc

            for f, segs in enumerate(fills):
                acc = psum_pool.tile([128, 2048], fp32, tag="ps",
                                     name=f"acc_{f}")
                offs, off = [], 0
                for (m, s, c0, w, _, _, _) in segs:
                    offs.append(off)
                    off += -(-w // 512) * 512
                for P in range(2):
                    for (m, s, c0, w, _, _, _), so in zip(segs, offs):
                        for o in range(0, w, 512):
                            wc = min(512, w - o)
                            nc.tensor.matmul(
                                acc[:, so + o:so + o + wc],
                                xnT4[0][:, P, :, m * 128:(m + 1) * 128],
                                xnT4[s][:, P, :, c0 + o:c0 + o + wc],
                                start=(P == 0), stop=(P == 1),
                                perf_mode=DR)
                ot = p2out.tile([128, 2048], bf16, tag="ot", name=f"ot_{f}")
                # Exp over gap-free runs of segments
                run_s, run_e = None, None
                for (m, s, c0, w, _, _, _), so in zip(segs, offs):
                    if run_e == so:
                        run_e = so + w
                    else:
                        if run_s is not None:
                            nc.scalar.activation(
                                ot[:, run_s:run_e], acc[:, run_s:run_e],
                                mybir.ActivationFunctionType.Exp, scale=-0.5)
                        run_s, run_e = so, so + w
                nc.scalar.activation(ot[:, run_s:run_e], acc[:, run_s:run_e],
                                     mybir.ActivationFunctionType.Exp,
                                     scale=-0.5)
                if segs[0][4] is outc:
                    # one DMA for the whole fill: [p, 2(m), 1024] layout
                    m0 = segs[0][0]
                    nc.sync.dma_start(outc.ap()[:, m0:m0 + 2, :],
                                      ot[:, 0:2048])
                else:
                    for (m, s, c0, w, dst, drow, dcol), so in zip(segs, offs):
                        nc.sync.dma_start(
                            dst.ap()[drow:drow + 128, dcol:dcol + w],
                            ot[:, so:so + w])

    nc.compile()
    return nc


def _pack(xn_rows):
    """[2048, 512] fp8 -> [128, 2, 2, 2048] DoubleRow layout, contiguous."""
    # xnT[d, col]; d = 256*P + 128*i + p  ->  [p, P, i, col]
    t = np.ascontiguousarray(
        xn_rows.T.reshape(2, 2, 128, R).transpose(2, 0, 1, 3))
    return t


def _in_maps(x):
    norm = np.sqrt(np.sum(x.astype(np.float64) ** 2, axis=-1, keepdims=True))
    xn = (x / np.maximum(norm, EPS)).astype(np.float32)
    xn8 = xn.astype(ml_dtypes.float8_e4m3fn)
    maps = []
    for c in range(N_CORES):
        b = c // 2
        xb = xn8[b]
        if c % 2 == 0:
            own, cross = xb[0:R], xb[R:N]
        else:
            own = xb[R:N]
            cross = np.concatenate([xb[Q:2 * Q], xb[0:Q]])
        maps.append({"xnt0": _pack(own), "xnt1": _pack(cross)})
    return maps


def _assemble(results, out):
    for c in range(N_CORES):
        b, odd = c // 2, c % 2
        o = out[b]
        r0 = odd * 2 * Q                  # own-row offset: 0 or 2048
        dtop = np.asarray(results[c]["dtop"]).astype(np.float32)
        dbot = np.asarray(results[c]["dbot"]).astype(np.float32)
        outc = np.asarray(results[c]["outc"]).astype(np.float32) \
            .transpose(1, 0, 2).reshape(2 * Q, Q)
        o[r0:r0 + Q, r0:r0 + 2 * Q] = dtop
        o[r0 + Q:r0 + 2 * Q, r0 + Q:r0 + 2 * Q] = dbot
        # mirror skipped lower-left 128-blocks inside the diagonal blocks
        for base in (r0, r0 + Q):
            for mm in range(1, 8):
                rr = base + mm * 128
                for cc in range(mm):
                    cb = base + cc * 128
                    o[rr:rr + 128, cb:cb + 128] = \
                        o[cb:cb + 128, rr:rr + 128].T
        o[r0 + Q:r0 + 2 * Q, r0:r0 + Q] = dtop[:, Q:2 * Q].T
        # cross cols: even core -> [2048.., 3072..]; odd -> [1024.., 0..]
        ccol = [2 * Q, 3 * Q] if not odd else [Q, 0]
        for half in range(2):
            blk = outc[half * Q:(half + 1) * Q]
            rr = r0 + half * Q
            cc = ccol[half]
            o[rr:rr + Q, cc:cc + Q] = blk
            o[cc:cc + Q, rr:rr + Q] = blk.T
    return out


def kernel(x: np.ndarray) -> np.ndarray:
    from concourse.bass_utils import run_bass_kernel_spmd

    x = np.asarray(x, dtype=np.float32)
    assert x.shape == (B, N, D)

    if "nc" not in _compiled:
        _compiled["nc"] = _build()
    nc = _compiled["nc"]

    res = run_bass_kernel_spmd(nc, _in_maps(x), list(range(N_CORES)))
    out = np.empty((B, N, N), dtype=np.float32)
    return _assemble([res.results[c] for c in range(N_CORES)], out)
